# revision 33
# baseline (speedup 1.0000x reference)
"""Multi-head attention (B=2, S=2048, D=1024, H=16) on 8 Trainium2 cores.

Sharding: core = (batch b in {0,1}) x (head-group g in {0..3}); each core
computes its 4 heads end to end plus a partial output projection that the
host sums (4 partials per batch) and biases.

v2 design: the softmax exp on the Activation engine is the hard floor
(~135us of back-to-back [128,1024] EXPs per core); everything else is
arranged to hide under it:
  - Q/K projections: fp8e4 DoubleRow matmuls (2 contraction chunks per
    instruction) off fp8 X^T / W.
  - Scores: fp8e4 DoubleRow with the d=64 contraction split into two
    32-row subtiles ([32, 2, .] operands); Q^T/K^T are evacuated to fp8
    and partition-remapped 128->32x4 by small gpsimd DMAs.
  - V path, P.V and output projection stay bf16 for accuracy.
  - Windows: (qc, p) pairs; window w runs scores+exp for w while P.V for
    w-1 drains at 2/kb (done by kb7), norm at kb8, then out-proj /
    projection fillers use the spare PE slots.
  - Softmax normalization: reciprocal of ones-column rowsums (DVE),
    partition_broadcast on gpsimd, two DVE muls.
  - Output projection DMAs straight from PSUM to DRAM.
"""

import ml_dtypes
import numpy as np

import concourse.bass as bass
import concourse.bacc as bacc
import concourse.mybir as mybir
import concourse.tile as tile
from concourse.bass_utils import run_bass_kernel_spmd

F32 = mybir.dt.float32
BF16 = mybir.dt.bfloat16
FP8 = mybir.dt.float8e4
AF = mybir.ActivationFunctionType
DR = mybir.MatmulPerfMode.DoubleRow

B = 2
S = 2048
D = 1024
H = 16
DK = 64
GH = 4            # heads per core
GD = GH * DK      # 256: projection slice width per core
SC = 256          # s-chunk for projections
NSC = S // SC     # 8
NDC = D // 128    # 8 contraction chunks
NDP = NDC // 2    # 4 DoubleRow contraction chunk-pairs
QC = 512          # q-chunk for attention
NQC = S // QC     # 4
NKB = S // 128    # 16 key blocks
SCALE = 1.0 / np.sqrt(np.float32(DK))

SCORES_FP8 = False   # fp8 DoubleRow scores (else bf16 2x64-contraction)


def build_nc():
    nc = bacc.Bacc()

    xqt = nc.dram_tensor("xqt", [NSC, 128, NDP, 2, SC], FP8, kind="ExternalInput")
    xkt = nc.dram_tensor("xkt", [NSC, 128, NDP, 2, SC], FP8, kind="ExternalInput")
    xvt = nc.dram_tensor("xvt", [NSC, 128, NDC, SC], BF16, kind="ExternalInput")
    wq = nc.dram_tensor("wq", [128, NDP, 2, 2, 128], FP8, kind="ExternalInput")
    wk = nc.dram_tensor("wk", [128, NDP, 2, 2, 128], FP8, kind="ExternalInput")
    wv = nc.dram_tensor("wv", [128, NDC, GD], BF16, kind="ExternalInput")
    wo = nc.dram_tensor("wo", [128, 2, D], BF16, kind="ExternalInput")
    bq = nc.dram_tensor("bq", [128, 2], F32, kind="ExternalInput")
    bk = nc.dram_tensor("bk", [128, 2], F32, kind="ExternalInput")
    bv = nc.dram_tensor("bv", [GD], F32, kind="ExternalInput")
    out = nc.dram_tensor("out", [S, D], F32, kind="ExternalOutput")

    with tile.TileContext(nc) as tc:
        with (
            tc.tile_pool(name="persist", bufs=1) as persist,
            tc.tile_pool(name="sxk", bufs=8) as sxk,
            tc.tile_pool(name="sxq", bufs=8) as sxq,
            tc.tile_pool(name="sxv", bufs=8) as sxv,
            tc.tile_pool(name="s8", bufs=4) as s8,
            tc.tile_pool(name="work", bufs=4) as work,
            tc.tile_pool(name="ptp", bufs=18) as ptp,
            tc.tile_pool(name="pst", bufs=2, space="PSUM") as pst,
            tc.tile_pool(name="ppv", bufs=1, space="PSUM") as ppv,
            tc.tile_pool(name="pproj", bufs=1, space="PSUM") as pproj,
            tc.tile_pool(name="pop", bufs=1, space="PSUM") as pop,
        ):
            # ---- weights / constants (small DMAs first) -------------------
            # small/critical weights first; wv before the first vproj and wo
            # before the first outproj go later in the queue
            wq_sb = persist.tile([128, NDP, 2, 2, 128], FP8, tag="wq_sb")
            wk_sb = persist.tile([128, NDP, 2, 2, 128], FP8, tag="wk_sb")
            wv_sb = persist.tile([128, NDC, GD], BF16, tag="wv_sb")
            wo_sb = persist.tile([128, 2, D], BF16, tag="wo_sb")

            bq_sb = persist.tile([128, 2], F32, tag="bq_sb")
            bk_sb = persist.tile([128, 2], F32, tag="bk_sb")
            bv_ap = bv[:]
            bv_bcast = persist.tile([128, GD], F32, tag="bv_bcast")
            nc.gpsimd.dma_start(
                out=bv_bcast,
                in_=bass.AP(tensor=bv_ap.tensor, offset=bv_ap.offset,
                            ap=[[0, 128]] + [list(p) for p in bv_ap.ap]),
            )

            # ---- persistent activations ----------------------------------
            if SCORES_FP8:
                # [32 part, d-half 2, head 4, S]
                qt8 = persist.tile([32, 2, GH, S], FP8, tag="qt8")
                kt8 = persist.tile([32, 2, GH, S], FP8, tag="kt8")
            else:
                qt_sb = persist.tile([128, 2, S], BF16, tag="qt_sb")
                kt_sb = persist.tile([128, 2, S], BF16, tag="kt_sb")
            vhat_sb = persist.tile([128, NKB, GH, DK + 1], BF16, tag="vhat_sb")
            nc.vector.memset(vhat_sb[:, :, :, DK:DK + 1], 1.0)      # ones column
            ones_sb = persist.tile([1, GD], BF16, tag="ones_sb")
            nc.vector.memset(ones_sb, 1.0)
            ot_sb = persist.tile([128, 2, S], BF16, tag="ot_sb")     # attn out^T

            # ---- input DMAs (single sync queue; order = priority) --------
            xk_t, xq_t, xv_t = [None] * NSC, [None] * NSC, [None] * NSC

            def dma_xk(sc):
                xk_t[sc] = sxk.tile([128, NDP, 2, SC], FP8, tag="xk", name="xk_t")
                nc.sync.dma_start(out=xk_t[sc], in_=xkt[sc])

            def dma_xq(sc):
                xq_t[sc] = sxq.tile([128, NDP, 2, SC], FP8, tag="xq", name="xq_t")
                nc.sync.dma_start(out=xq_t[sc], in_=xqt[sc])

            def dma_xv(sc):
                xv_t[sc] = sxv.tile([128, NDC, SC], BF16, tag="xv", name="xv_t")
                nc.sync.dma_start(out=xv_t[sc], in_=xvt[sc])

            nc.sync.dma_start(out=wk_sb, in_=wk[:, :, :, :, :])
            dma_xk(0); dma_xk(1)
            nc.sync.dma_start(out=wq_sb, in_=wq[:, :, :, :, :])
            dma_xq(0); dma_xq(1)
            nc.sync.dma_start(out=bk_sb, in_=bk[:, :])
            nc.sync.dma_start(out=bq_sb, in_=bq[:, :])
            for sc in range(2, NSC):
                dma_xk(sc)
            nc.sync.dma_start(out=wv_sb, in_=wv[:, :, :])
            for sc in range(NSC):
                dma_xv(sc)
            for sc in range(2, NSC):
                dma_xq(sc)
            nc.sync.dma_start(out=wo_sb, in_=wo[:, :, :])

            # ---- projections ---------------------------------------------
            def emit_qkproj_group(sc, x_t, w_sb, b_sb, dst8, c, alt=False):
                # fp8 DoubleRow proj; alt=True borrows the (idle) pop bank
                # during the lead so groups double-buffer
                ss = bass.ts(sc, SC)
                if True:
                    if alt:
                        ps = pop.tile([128, 512], F32, tag="op",
                                      name="ps_alt")[:, 0:SC]
                    else:
                        ps = pproj.tile([128, SC], F32, tag="ps_proj")
                    for dp in range(NDP):
                        nc.tensor.matmul(
                            ps,
                            lhsT=w_sb[:, dp, :, c, :],
                            rhs=x_t[:, dp, :, :],
                            start=(dp == 0), stop=(dp == NDP - 1),
                            perf_mode=DR,
                        )
                    if SCORES_FP8:
                        st8 = s8.tile([128, SC], FP8, tag="st8")
                        nc.vector.tensor_scalar_add(
                            out=st8, in0=ps, scalar1=b_sb[:, c:c + 1]
                        )
                        for hl in range(2):
                            for t in range(2):
                                p0 = hl * 64 + t * 32
                                nc.gpsimd.dma_start(
                                    out=dst8[0:32, t, 2 * c + hl, ss],
                                    in_=st8[p0:p0 + 32, :],
                                )
                    else:
                        nc.vector.tensor_scalar_add(
                            out=dst8[:, c, ss], in0=ps, scalar1=b_sb[:, c:c + 1]
                        )

            _alt = [0]

            def emit_qkproj(sc, x_t, w_sb, b_sb, dst8):
                for c in range(2):
                    _alt[0] ^= 1
                    emit_qkproj_group(sc, x_t, w_sb, b_sb, dst8, c,
                                      alt=bool(_alt[0]))

            def emit_vproj_half(sc, half, alt=False):
                if True:
                    kb = sc * (SC // 128) + half
                    if alt:
                        ps = pop.tile([128, 512], F32, tag="op",
                                      name="ps_alt")[:, 0:GD]
                    else:
                        ps = pproj.tile([128, GD], F32, tag="ps_proj")
                    for dc in range(NDC):
                        nc.tensor.matmul(
                            ps,
                            lhsT=xv_t[sc][:, dc, bass.ts(half, 128)],
                            rhs=wv_sb[:, dc, :],
                            start=(dc == 0), stop=(dc == NDC - 1),
                        )
                    nc.vector.tensor_add(
                        out=vhat_sb[:, kb, :, 0:DK],
                        in0=ps.rearrange("p (h d) -> p h d", h=GH),
                        in1=bv_bcast.rearrange("p (h d) -> p h d", h=GH),
                    )

            def emit_vproj(sc):
                for half in range(SC // 128):
                    _alt[0] ^= 1
                    emit_vproj_half(sc, half, alt=bool(_alt[0]))

            # ---- attention -----------------------------------------------
            pt_tiles = {}
            pv_acc = {}
            pv_done = {}

            def emit_scores_exp(qc, p, kb):
                qs = bass.ts(qc, QC)
                ks = bass.ts(kb, 128)
                st = pst.tile([128, 2 * QC], F32, tag="st")
                if SCORES_FP8:
                    for i in range(2):
                        h = 2 * p + i
                        nc.tensor.matmul(
                            st[:, i * QC:(i + 1) * QC],
                            lhsT=kt8[:, :, h, ks],
                            rhs=qt8[:, :, h, qs],
                            start=True, stop=True,
                            perf_mode=DR,
                        )
                else:
                    nc.tensor.matmul(
                        st[:, 0:QC], lhsT=kt_sb[0:64, p, ks],
                        rhs=qt_sb[0:64, p, qs], start=True, stop=True,
                    )
                    nc.tensor.matmul(
                        st[:, QC:2 * QC], lhsT=kt_sb[64:128, p, ks],
                        rhs=qt_sb[64:128, p, qs],
                        start=True, stop=True, tile_position=(64, 0),
                    )
                pt = ptp.tile([128, 2 * QC], BF16, tag="pt")
                nc.scalar.activation(pt, st, AF.Exp, scale=float(SCALE))
                pt_tiles[(qc, p, kb)] = pt

            def pump_pv(qc, p, upto):
                n = pv_done.get((qc, p), 0)
                while n < min(upto, NKB):
                    kb = n
                    if (qc, p) not in pv_acc:
                        pv_acc[(qc, p)] = ppv.tile([65, 2, QC], F32, tag="pv", name="pv")
                    pv = pv_acc[(qc, p)]
                    pt = pt_tiles.pop((qc, p, kb))
                    for i in range(2):
                        nc.tensor.matmul(
                            pv[:, i, :], lhsT=vhat_sb[:, kb, 2 * p + i, :],
                            rhs=pt[:, i * QC:(i + 1) * QC],
                            start=(kb == 0), stop=(kb == NKB - 1),
                        )
                    n += 1
                pv_done[(qc, p)] = n

            def emit_norm(qc, p):
                qs = bass.ts(qc, QC)
                pv = pv_acc.pop((qc, p))
                rs = work.tile([1, 2, QC], F32, tag="rs")
                nc.vector.tensor_copy(rs, pv[64:65, :, :])   # custom-DVE ops
                rr = work.tile([1, 2, QC], F32, tag="rr")    # can't read PSUM
                nc.vector.reciprocal_approx_fast(out=rr, in_=rs)
                rrb = work.tile([1, 2, QC], BF16, tag="rrb")
                nc.vector.tensor_copy(rrb, rr)
                for i in range(2):
                    # broadcast 1/rowsum across 64 partitions via PE outer
                    # product (shares the out-proj psum bank)
                    bct = pop.tile([128, 512], F32, tag="op", name="bct")
                    nc.tensor.matmul(
                        bct[0:64, :], lhsT=ones_sb[0:1, 0:64], rhs=rrb[0:1, i, :],
                        start=True, stop=True,
                    )
                    bcs = work.tile([64, 512], F32, tag="bcs")
                    nc.vector.tensor_copy(bcs, bct[0:64, :])
                    nc.vector.tensor_mul(
                        ot_sb[64 * i:64 * (i + 1), p, qs], pv[0:64, i, :], bcs,
                    )

            def emit_outproj_group(qc, qb, dm, alt=False):
                qbs = bass.ts(qc * (QC // 128) + qb, 128)
                if alt:
                    # tail only: borrow a (now idle) score-psum tile so the
                    # single pop bank doesn't serialize the final groups
                    op = pst.tile([128, 2 * QC], F32, tag="st", name="op_alt")[:, 0:512]
                else:
                    op = pop.tile([128, 512], F32, tag="op")
                for c in range(2):
                    nc.tensor.matmul(
                        op,
                        lhsT=ot_sb[:, c, qbs],
                        rhs=wo_sb[:, c, bass.ts(dm, 512)],
                        start=(c == 0), stop=(c == 1),
                    )
                obuf = work.tile([128, 512], F32, tag="obuf")
                nc.vector.tensor_copy(obuf, op)
                row0 = (qc * (QC // 128) + qb) * 128
                nc.sync.dma_start(
                    out=out[row0:row0 + 128, bass.ts(dm, 512)], in_=obuf,
                )

            # ================= emission schedule ==========================
            kdst = kt8 if SCORES_FP8 else kt_sb
            qdst = qt8 if SCORES_FP8 else qt_sb
            emit_qkproj(0, xk_t[0], wk_sb, bk_sb, kdst)
            emit_qkproj(1, xk_t[1], wk_sb, bk_sb, kdst)
            emit_qkproj(0, xq_t[0], wq_sb, bq_sb, qdst)
            emit_qkproj(1, xq_t[1], wq_sb, bq_sb, qdst)
            scored = 0

            def pump_scores(qc, p, upto):
                nonlocal scored
                while scored < min(upto, NKB):
                    emit_scores_exp(qc, p, scored)
                    scored += 1

            for sc in range(2, NSC):
                emit_qkproj(sc, xk_t[sc], wk_sb, bk_sb, kdst)
                pump_scores(0, 0, 2 * sc - 2)
            pump_scores(0, 0, NKB)
            for sc in range(6):
                emit_vproj(sc)

            # filler queue: one-psum-group closures consumed in spare PE
            # slots (kb odd) so they never delay the next scores matmul much
            def qproj_filler(sc, c):
                return lambda: emit_qkproj_group(sc, xq_t[sc], wq_sb, bq_sb,
                                                 qdst, c)

            def outproj_filler(qc, g):
                return lambda: emit_outproj_group(qc, g // 2, g % 2)

            fillers = [lambda: emit_vproj_half(6, 0), lambda: emit_vproj_half(6, 1),
                       lambda: emit_vproj_half(7, 0), lambda: emit_vproj_half(7, 1)]
            fillers += [qproj_filler(sc, c) for sc in (2, 3) for c in (0, 1)]
            pending_qproj = [qproj_filler(sc, c) for sc in range(4, NSC)
                             for c in (0, 1)]

            windows = [(qc, p) for qc in range(NQC) for p in range(2)]
            for w in range(1, len(windows)):
                qc, p = windows[w]
                cqc, cp = windows[w - 1]          # chase window
                for _ in range(2):
                    if pending_qproj:
                        fillers.append(pending_qproj.pop(0))
                for kb in range(NKB):
                    # fillers go FIRST in the slot: they may produce data
                    # (vhat/qt) that this slot's P.V pump reads
                    if kb % 2 == 1 and fillers:
                        fillers.pop(0)()
                    emit_scores_exp(qc, p, kb)
                    pump_pv(cqc, cp, 2 * kb)
                    if kb == 8:
                        pump_pv(cqc, cp, NKB)
                        emit_norm(cqc, cp)
                        if cp == 1:
                            for g in range(8):
                                fillers.append(outproj_filler(cqc, g))
                    if w == len(windows) - 1 and kb >= 9:
                        pump_pv(qc, p, min(3 * (kb - 8), kb + 1))

            # ---- tail ----------------------------------------------------
            qc, p = windows[-1]
            pump_pv(qc, p, NKB)
            emit_norm(qc, p)
            while fillers:
                fillers.pop(0)()
            for g in range(8):
                emit_outproj_group(NQC - 1, g // 2, g % 2, alt=(g % 3 != 0))
    return nc


_NC_CACHE = None


def _get_nc():
    global _NC_CACHE
    if _NC_CACHE is None:
        nc = build_nc()
        nc.finalize()
        _NC_CACHE = nc
    return _NC_CACHE


def _prep_xt8(x):
    # [S, D] -> X^T [NSC, 128, NDP, 2, SC] fp8e4
    xt = x.T.astype(ml_dtypes.float8_e4m3)              # [D, S]
    return np.ascontiguousarray(
        xt.reshape(NDP, 2, 128, NSC, SC).transpose(3, 2, 0, 1, 4)
    )


def _prep_xt(x):
    # [S, D] -> X^T [NSC, 128, NDC, SC] bf16
    xt = x.T.astype(ml_dtypes.bfloat16)                 # [D, S]
    return np.ascontiguousarray(
        xt.reshape(NDC, 128, NSC, SC).transpose(2, 1, 0, 3)
    )


def _prep_w8(w):
    # [1024, GD] -> [128, NDP, 2, 2, 128] fp8e4
    return np.ascontiguousarray(
        w.astype(ml_dtypes.float8_e4m3)
        .reshape(NDP, 2, 128, 2, 128).transpose(2, 0, 1, 3, 4))


def _prep_w(w):
    # [1024, GD] -> [128, NDC, GD] bf16
    return np.ascontiguousarray(
        w.astype(ml_dtypes.bfloat16).reshape(NDC, 128, GD).transpose(1, 0, 2))


def _prep_wo(w):
    # [GD, 1024] -> [128, 2, 1024] bf16
    return np.ascontiguousarray(
        w.astype(ml_dtypes.bfloat16).reshape(2, 128, D).transpose(1, 0, 2))


def kernel(q, k, v, Wq, bq, Wk, bk, Wv, bv, Wo, bo):
    q = np.asarray(q, np.float32)
    k = np.asarray(k, np.float32)
    v = np.asarray(v, np.float32)
    Wq = np.asarray(Wq, np.float32)
    Wk = np.asarray(Wk, np.float32)
    Wv = np.asarray(Wv, np.float32)
    Wo = np.asarray(Wo, np.float32)
    bq = np.asarray(bq, np.float32)
    bk = np.asarray(bk, np.float32)
    bv = np.asarray(bv, np.float32)
    bo = np.asarray(bo, np.float32)

    nc = _get_nc()

    xqt = [_prep_xt8(q[b]) for b in range(B)]
    xkt = [_prep_xt8(k[b]) for b in range(B)]
    xvt = [_prep_xt(v[b]) for b in range(B)]

    in_maps = []
    for core in range(8):
        b, g = divmod(core, 4)
        gs = slice(g * GD, (g + 1) * GD)
        in_maps.append({
            "xqt": xqt[b], "xkt": xkt[b], "xvt": xvt[b],
            "wq": _prep_w8(Wq[:, gs]),
            "wk": _prep_w8(Wk[:, gs]),
            "wv": _prep_w(Wv[:, gs]),
            "wo": _prep_wo(Wo[gs, :]),
            "bq": np.ascontiguousarray(bq[gs].reshape(2, 128).T),
            "bk": np.ascontiguousarray(bk[gs].reshape(2, 128).T),
            "bv": np.ascontiguousarray(bv[gs]),
        })

    res = run_bass_kernel_spmd(nc, in_maps, core_ids=list(range(8)))

    out = np.empty((B, S, D), np.float32)
    for b in range(B):
        acc = res.results[4 * b]["out"].astype(np.float32).copy()
        for g in range(1, 4):
            acc += res.results[4 * b + g]["out"]
        out[b] = acc + bo
    return out


# revision 34
# speedup vs baseline: 1.0056x; 1.0056x over previous
"""Multi-head attention (B=2, S=2048, D=1024, H=16) on 8 Trainium2 cores.

Sharding: core = (batch b in {0,1}) x (head-group g in {0..3}); each core
computes its 4 heads end to end plus a partial output projection that the
host sums (4 partials per batch) and biases.

v2 design: the softmax exp on the Activation engine is the hard floor
(~135us of back-to-back [128,1024] EXPs per core); everything else is
arranged to hide under it:
  - Q/K projections: fp8e4 DoubleRow matmuls (2 contraction chunks per
    instruction) off fp8 X^T / W.
  - Scores: fp8e4 DoubleRow with the d=64 contraction split into two
    32-row subtiles ([32, 2, .] operands); Q^T/K^T are evacuated to fp8
    and partition-remapped 128->32x4 by small gpsimd DMAs.
  - V path, P.V and output projection stay bf16 for accuracy.
  - Windows: (qc, p) pairs; window w runs scores+exp for w while P.V for
    w-1 drains at 2/kb (done by kb7), norm at kb8, then out-proj /
    projection fillers use the spare PE slots.
  - Softmax normalization: reciprocal of ones-column rowsums (DVE),
    partition_broadcast on gpsimd, two DVE muls.
  - Output projection DMAs straight from PSUM to DRAM.
"""

import ml_dtypes
import numpy as np

import concourse.bass as bass
import concourse.bacc as bacc
import concourse.mybir as mybir
import concourse.tile as tile
from concourse.bass_utils import run_bass_kernel_spmd

F32 = mybir.dt.float32
BF16 = mybir.dt.bfloat16
FP8 = mybir.dt.float8e4
AF = mybir.ActivationFunctionType
DR = mybir.MatmulPerfMode.DoubleRow

B = 2
S = 2048
D = 1024
H = 16
DK = 64
GH = 4            # heads per core
GD = GH * DK      # 256: projection slice width per core
SC = 256          # s-chunk for projections
NSC = S // SC     # 8
NDC = D // 128    # 8 contraction chunks
NDP = NDC // 2    # 4 DoubleRow contraction chunk-pairs
QC = 512          # q-chunk for attention
NQC = S // QC     # 4
NKB = S // 128    # 16 key blocks
SCALE = 1.0 / np.sqrt(np.float32(DK))

SCORES_FP8 = False   # fp8 DoubleRow scores (else bf16 2x64-contraction)


def build_nc():
    nc = bacc.Bacc()

    xqt = nc.dram_tensor("xqt", [NSC, 128, NDP, 2, SC], FP8, kind="ExternalInput")
    xkt = nc.dram_tensor("xkt", [NSC, 128, NDP, 2, SC], FP8, kind="ExternalInput")
    xvt = nc.dram_tensor("xvt", [NSC, 128, NDC, SC], BF16, kind="ExternalInput")
    wq = nc.dram_tensor("wq", [128, NDP, 2, 2, 128], FP8, kind="ExternalInput")
    wk = nc.dram_tensor("wk", [128, NDP, 2, 2, 128], FP8, kind="ExternalInput")
    wv = nc.dram_tensor("wv", [128, NDC, GD], BF16, kind="ExternalInput")
    wo = nc.dram_tensor("wo", [128, 2, D], BF16, kind="ExternalInput")
    bq = nc.dram_tensor("bq", [128, 2], F32, kind="ExternalInput")
    bk = nc.dram_tensor("bk", [128, 2], F32, kind="ExternalInput")
    bv = nc.dram_tensor("bv", [GD], F32, kind="ExternalInput")
    out = nc.dram_tensor("out", [S, D], F32, kind="ExternalOutput")

    with tile.TileContext(nc) as tc:
        with (
            tc.tile_pool(name="persist", bufs=1) as persist,
            tc.tile_pool(name="sxk", bufs=8) as sxk,
            tc.tile_pool(name="sxq", bufs=8) as sxq,
            tc.tile_pool(name="sxv", bufs=8) as sxv,
            tc.tile_pool(name="s8", bufs=4) as s8,
            tc.tile_pool(name="work", bufs=4) as work,
            tc.tile_pool(name="ptp", bufs=18) as ptp,
            tc.tile_pool(name="pst", bufs=2, space="PSUM") as pst,
            tc.tile_pool(name="ppv", bufs=1, space="PSUM") as ppv,
            tc.tile_pool(name="pproj", bufs=1, space="PSUM") as pproj,
            tc.tile_pool(name="pop", bufs=1, space="PSUM") as pop,
        ):
            # ---- weights / constants (small DMAs first) -------------------
            # small/critical weights first; wv before the first vproj and wo
            # before the first outproj go later in the queue
            wq_sb = persist.tile([128, NDP, 2, 2, 128], FP8, tag="wq_sb")
            wk_sb = persist.tile([128, NDP, 2, 2, 128], FP8, tag="wk_sb")
            wv_sb = persist.tile([128, NDC, GD], BF16, tag="wv_sb")
            wo_sb = persist.tile([128, 2, D], BF16, tag="wo_sb")

            bq_sb = persist.tile([128, 2], F32, tag="bq_sb")
            bk_sb = persist.tile([128, 2], F32, tag="bk_sb")
            nc.gpsimd.dma_start(out=wk_sb, in_=wk[:, :, :, :, :])
            nc.gpsimd.dma_start(out=wq_sb, in_=wq[:, :, :, :, :])
            bv_ap = bv[:]
            bv_bcast = persist.tile([128, GD], F32, tag="bv_bcast")
            nc.gpsimd.dma_start(
                out=bv_bcast,
                in_=bass.AP(tensor=bv_ap.tensor, offset=bv_ap.offset,
                            ap=[[0, 128]] + [list(p) for p in bv_ap.ap]),
            )

            # ---- persistent activations ----------------------------------
            if SCORES_FP8:
                # [32 part, d-half 2, head 4, S]
                qt8 = persist.tile([32, 2, GH, S], FP8, tag="qt8")
                kt8 = persist.tile([32, 2, GH, S], FP8, tag="kt8")
            else:
                qt_sb = persist.tile([128, 2, S], BF16, tag="qt_sb")
                kt_sb = persist.tile([128, 2, S], BF16, tag="kt_sb")
            vhat_sb = persist.tile([128, NKB, GH, DK + 1], BF16, tag="vhat_sb")
            nc.vector.memset(vhat_sb[:, :, :, DK:DK + 1], 1.0)      # ones column
            ones_sb = persist.tile([1, GD], BF16, tag="ones_sb")
            nc.vector.memset(ones_sb, 1.0)
            ot_sb = persist.tile([128, 2, S], BF16, tag="ot_sb")     # attn out^T

            # ---- input DMAs (single sync queue; order = priority) --------
            xk_t, xq_t, xv_t = [None] * NSC, [None] * NSC, [None] * NSC

            def dma_xk(sc):
                xk_t[sc] = sxk.tile([128, NDP, 2, SC], FP8, tag="xk", name="xk_t")
                nc.sync.dma_start(out=xk_t[sc], in_=xkt[sc])

            def dma_xq(sc, eng=None):
                xq_t[sc] = sxq.tile([128, NDP, 2, SC], FP8, tag="xq", name="xq_t")
                (eng or nc.sync).dma_start(out=xq_t[sc], in_=xqt[sc])

            def dma_xv(sc):
                xv_t[sc] = sxv.tile([128, NDC, SC], BF16, tag="xv", name="xv_t")
                nc.sync.dma_start(out=xv_t[sc], in_=xvt[sc])

            dma_xk(0); dma_xk(1)
            dma_xq(0, nc.scalar); dma_xq(1, nc.scalar)
            nc.sync.dma_start(out=bk_sb, in_=bk[:, :])
            nc.sync.dma_start(out=bq_sb, in_=bq[:, :])
            for sc in range(2, NSC):
                dma_xk(sc)

            nc.sync.dma_start(out=wv_sb, in_=wv[:, :, :])
            for sc in range(NSC):
                dma_xv(sc)
            for sc in range(2, NSC):
                dma_xq(sc)
            nc.sync.dma_start(out=wo_sb, in_=wo[:, :, :])

            # ---- projections ---------------------------------------------
            def emit_qkproj_group(sc, x_t, w_sb, b_sb, dst8, c, alt=False):
                # fp8 DoubleRow proj; alt=True borrows the (idle) pop bank
                # during the lead so groups double-buffer
                ss = bass.ts(sc, SC)
                if True:
                    if alt:
                        ps = pop.tile([128, 512], F32, tag="op",
                                      name="ps_alt")[:, 0:SC]
                    else:
                        ps = pproj.tile([128, SC], F32, tag="ps_proj")
                    for dp in range(NDP):
                        nc.tensor.matmul(
                            ps,
                            lhsT=w_sb[:, dp, :, c, :],
                            rhs=x_t[:, dp, :, :],
                            start=(dp == 0), stop=(dp == NDP - 1),
                            perf_mode=DR,
                        )
                    if SCORES_FP8:
                        st8 = s8.tile([128, SC], FP8, tag="st8")
                        nc.vector.tensor_scalar_add(
                            out=st8, in0=ps, scalar1=b_sb[:, c:c + 1]
                        )
                        for hl in range(2):
                            for t in range(2):
                                p0 = hl * 64 + t * 32
                                nc.gpsimd.dma_start(
                                    out=dst8[0:32, t, 2 * c + hl, ss],
                                    in_=st8[p0:p0 + 32, :],
                                )
                    else:
                        nc.vector.tensor_scalar_add(
                            out=dst8[:, c, ss], in0=ps, scalar1=b_sb[:, c:c + 1]
                        )

            _alt = [0]

            def emit_qkproj(sc, x_t, w_sb, b_sb, dst8):
                for c in range(2):
                    _alt[0] ^= 1
                    emit_qkproj_group(sc, x_t, w_sb, b_sb, dst8, c,
                                      alt=bool(_alt[0]))

            def emit_vproj_half(sc, half, alt=False):
                if True:
                    kb = sc * (SC // 128) + half
                    if alt:
                        ps = pop.tile([128, 512], F32, tag="op",
                                      name="ps_alt")[:, 0:GD]
                    else:
                        ps = pproj.tile([128, GD], F32, tag="ps_proj")
                    for dc in range(NDC):
                        nc.tensor.matmul(
                            ps,
                            lhsT=xv_t[sc][:, dc, bass.ts(half, 128)],
                            rhs=wv_sb[:, dc, :],
                            start=(dc == 0), stop=(dc == NDC - 1),
                        )
                    nc.vector.tensor_add(
                        out=vhat_sb[:, kb, :, 0:DK],
                        in0=ps.rearrange("p (h d) -> p h d", h=GH),
                        in1=bv_bcast.rearrange("p (h d) -> p h d", h=GH),
                    )

            def emit_vproj(sc):
                for half in range(SC // 128):
                    _alt[0] ^= 1
                    emit_vproj_half(sc, half, alt=bool(_alt[0]))

            # ---- attention -----------------------------------------------
            pt_tiles = {}
            pv_acc = {}
            pv_done = {}

            def emit_scores_exp(qc, p, kb):
                qs = bass.ts(qc, QC)
                ks = bass.ts(kb, 128)
                st = pst.tile([128, 2 * QC], F32, tag="st")
                if SCORES_FP8:
                    for i in range(2):
                        h = 2 * p + i
                        nc.tensor.matmul(
                            st[:, i * QC:(i + 1) * QC],
                            lhsT=kt8[:, :, h, ks],
                            rhs=qt8[:, :, h, qs],
                            start=True, stop=True,
                            perf_mode=DR,
                        )
                else:
                    nc.tensor.matmul(
                        st[:, 0:QC], lhsT=kt_sb[0:64, p, ks],
                        rhs=qt_sb[0:64, p, qs], start=True, stop=True,
                    )
                    nc.tensor.matmul(
                        st[:, QC:2 * QC], lhsT=kt_sb[64:128, p, ks],
                        rhs=qt_sb[64:128, p, qs],
                        start=True, stop=True, tile_position=(64, 0),
                    )
                pt = ptp.tile([128, 2 * QC], BF16, tag="pt")
                nc.scalar.activation(pt, st, AF.Exp, scale=float(SCALE))
                pt_tiles[(qc, p, kb)] = pt

            def pump_pv(qc, p, upto):
                n = pv_done.get((qc, p), 0)
                while n < min(upto, NKB):
                    kb = n
                    if (qc, p) not in pv_acc:
                        pv_acc[(qc, p)] = ppv.tile([65, 2, QC], F32, tag="pv", name="pv")
                    pv = pv_acc[(qc, p)]
                    pt = pt_tiles.pop((qc, p, kb))
                    for i in range(2):
                        nc.tensor.matmul(
                            pv[:, i, :], lhsT=vhat_sb[:, kb, 2 * p + i, :],
                            rhs=pt[:, i * QC:(i + 1) * QC],
                            start=(kb == 0), stop=(kb == NKB - 1),
                        )
                    n += 1
                pv_done[(qc, p)] = n

            def emit_norm(qc, p):
                qs = bass.ts(qc, QC)
                pv = pv_acc.pop((qc, p))
                rs = work.tile([1, 2, QC], F32, tag="rs")
                nc.vector.tensor_copy(rs, pv[64:65, :, :])   # custom-DVE ops
                rr = work.tile([1, 2, QC], F32, tag="rr")    # can't read PSUM
                nc.vector.reciprocal_approx_fast(out=rr, in_=rs)
                rrb = work.tile([1, 2, QC], BF16, tag="rrb")
                nc.vector.tensor_copy(rrb, rr)
                for i in range(2):
                    # broadcast 1/rowsum across 64 partitions via PE outer
                    # product (shares the out-proj psum bank)
                    bct = pop.tile([128, 512], F32, tag="op", name="bct")
                    nc.tensor.matmul(
                        bct[0:64, :], lhsT=ones_sb[0:1, 0:64], rhs=rrb[0:1, i, :],
                        start=True, stop=True,
                    )
                    bcs = work.tile([64, 512], F32, tag="bcs")
                    nc.vector.tensor_copy(bcs, bct[0:64, :])
                    nc.vector.tensor_mul(
                        ot_sb[64 * i:64 * (i + 1), p, qs], pv[0:64, i, :], bcs,
                    )

            def emit_outproj_group(qc, qb, dm, alt=False):
                qbs = bass.ts(qc * (QC // 128) + qb, 128)
                if alt:
                    # tail only: borrow a (now idle) score-psum tile so the
                    # single pop bank doesn't serialize the final groups
                    op = pst.tile([128, 2 * QC], F32, tag="st", name="op_alt")[:, 0:512]
                else:
                    op = pop.tile([128, 512], F32, tag="op")
                for c in range(2):
                    nc.tensor.matmul(
                        op,
                        lhsT=ot_sb[:, c, qbs],
                        rhs=wo_sb[:, c, bass.ts(dm, 512)],
                        start=(c == 0), stop=(c == 1),
                    )
                obuf = work.tile([128, 512], F32, tag="obuf")
                nc.vector.tensor_copy(obuf, op)
                row0 = (qc * (QC // 128) + qb) * 128
                nc.sync.dma_start(
                    out=out[row0:row0 + 128, bass.ts(dm, 512)], in_=obuf,
                )

            # ================= emission schedule ==========================
            kdst = kt8 if SCORES_FP8 else kt_sb
            qdst = qt8 if SCORES_FP8 else qt_sb
            emit_qkproj(0, xk_t[0], wk_sb, bk_sb, kdst)
            emit_qkproj(1, xk_t[1], wk_sb, bk_sb, kdst)
            emit_qkproj(0, xq_t[0], wq_sb, bq_sb, qdst)
            emit_qkproj(1, xq_t[1], wq_sb, bq_sb, qdst)
            scored = 0

            def pump_scores(qc, p, upto):
                nonlocal scored
                while scored < min(upto, NKB):
                    emit_scores_exp(qc, p, scored)
                    scored += 1

            for sc in range(2, NSC):
                emit_qkproj(sc, xk_t[sc], wk_sb, bk_sb, kdst)
                pump_scores(0, 0, 2 * sc - 2)
            pump_scores(0, 0, 12)
            emit_vproj(0); emit_vproj(1); emit_vproj(2)
            pump_scores(0, 0, 14)
            emit_vproj(3); emit_vproj(4)
            pump_scores(0, 0, NKB)
            emit_vproj(5)

            # filler queue: one-psum-group closures consumed in spare PE
            # slots (kb odd) so they never delay the next scores matmul much
            def qproj_filler(sc, c):
                return lambda: emit_qkproj_group(sc, xq_t[sc], wq_sb, bq_sb,
                                                 qdst, c)

            def outproj_filler(qc, g):
                return lambda: emit_outproj_group(qc, g // 2, g % 2)

            fillers = [lambda: emit_vproj_half(6, 0), lambda: emit_vproj_half(6, 1),
                       lambda: emit_vproj_half(7, 0), lambda: emit_vproj_half(7, 1)]
            fillers += [qproj_filler(sc, c) for sc in (2, 3) for c in (0, 1)]
            pending_qproj = [qproj_filler(sc, c) for sc in range(4, NSC)
                             for c in (0, 1)]

            windows = [(qc, p) for qc in range(NQC) for p in range(2)]
            for w in range(1, len(windows)):
                qc, p = windows[w]
                cqc, cp = windows[w - 1]          # chase window
                for _ in range(2):
                    if pending_qproj:
                        fillers.append(pending_qproj.pop(0))
                for kb in range(NKB):
                    # fillers go FIRST in the slot: they may produce data
                    # (vhat/qt) that this slot's P.V pump reads
                    if kb % 2 == 1 and fillers:
                        fillers.pop(0)()
                    emit_scores_exp(qc, p, kb)
                    pump_pv(cqc, cp, 2 * kb)
                    if kb == 8:
                        pump_pv(cqc, cp, NKB)
                        emit_norm(cqc, cp)
                        if cp == 1:
                            for g in range(8):
                                fillers.append(outproj_filler(cqc, g))
                    if w == len(windows) - 1 and kb >= 9:
                        pump_pv(qc, p, min(3 * (kb - 8), kb + 1))

            # ---- tail ----------------------------------------------------
            qc, p = windows[-1]
            pump_pv(qc, p, NKB)
            emit_norm(qc, p)
            while fillers:
                fillers.pop(0)()
            for g in range(8):
                emit_outproj_group(NQC - 1, g // 2, g % 2, alt=(g % 3 != 0))
    return nc


_NC_CACHE = None


def _get_nc():
    global _NC_CACHE
    if _NC_CACHE is None:
        nc = build_nc()
        nc.finalize()
        _NC_CACHE = nc
    return _NC_CACHE


def _prep_xt8(x):
    # [S, D] -> X^T [NSC, 128, NDP, 2, SC] fp8e4
    xt = x.T.astype(ml_dtypes.float8_e4m3)              # [D, S]
    return np.ascontiguousarray(
        xt.reshape(NDP, 2, 128, NSC, SC).transpose(3, 2, 0, 1, 4)
    )


def _prep_xt(x):
    # [S, D] -> X^T [NSC, 128, NDC, SC] bf16
    xt = x.T.astype(ml_dtypes.bfloat16)                 # [D, S]
    return np.ascontiguousarray(
        xt.reshape(NDC, 128, NSC, SC).transpose(2, 1, 0, 3)
    )


def _prep_w8(w):
    # [1024, GD] -> [128, NDP, 2, 2, 128] fp8e4
    return np.ascontiguousarray(
        w.astype(ml_dtypes.float8_e4m3)
        .reshape(NDP, 2, 128, 2, 128).transpose(2, 0, 1, 3, 4))


def _prep_w(w):
    # [1024, GD] -> [128, NDC, GD] bf16
    return np.ascontiguousarray(
        w.astype(ml_dtypes.bfloat16).reshape(NDC, 128, GD).transpose(1, 0, 2))


def _prep_wo(w):
    # [GD, 1024] -> [128, 2, 1024] bf16
    return np.ascontiguousarray(
        w.astype(ml_dtypes.bfloat16).reshape(2, 128, D).transpose(1, 0, 2))


def kernel(q, k, v, Wq, bq, Wk, bk, Wv, bv, Wo, bo):
    q = np.asarray(q, np.float32)
    k = np.asarray(k, np.float32)
    v = np.asarray(v, np.float32)
    Wq = np.asarray(Wq, np.float32)
    Wk = np.asarray(Wk, np.float32)
    Wv = np.asarray(Wv, np.float32)
    Wo = np.asarray(Wo, np.float32)
    bq = np.asarray(bq, np.float32)
    bk = np.asarray(bk, np.float32)
    bv = np.asarray(bv, np.float32)
    bo = np.asarray(bo, np.float32)

    nc = _get_nc()

    xqt = [_prep_xt8(q[b]) for b in range(B)]
    xkt = [_prep_xt8(k[b]) for b in range(B)]
    xvt = [_prep_xt(v[b]) for b in range(B)]

    in_maps = []
    for core in range(8):
        b, g = divmod(core, 4)
        gs = slice(g * GD, (g + 1) * GD)
        in_maps.append({
            "xqt": xqt[b], "xkt": xkt[b], "xvt": xvt[b],
            "wq": _prep_w8(Wq[:, gs]),
            "wk": _prep_w8(Wk[:, gs]),
            "wv": _prep_w(Wv[:, gs]),
            "wo": _prep_wo(Wo[gs, :]),
            "bq": np.ascontiguousarray(bq[gs].reshape(2, 128).T),
            "bk": np.ascontiguousarray(bk[gs].reshape(2, 128).T),
            "bv": np.ascontiguousarray(bv[gs]),
        })

    res = run_bass_kernel_spmd(nc, in_maps, core_ids=list(range(8)))

    out = np.empty((B, S, D), np.float32)
    for b in range(B):
        acc = res.results[4 * b]["out"].astype(np.float32).copy()
        for g in range(1, 4):
            acc += res.results[4 * b + g]["out"]
        out[b] = acc + bo
    return out
